# revision 6
# baseline (speedup 1.0000x reference)
"""Trainium2 Bass kernel for nn_Capsule: capsule routing head.

Math: the einsum 'nco,pbo->bno' factorizes as xp[b,n,o] = W[n,o] * X[b,o]
with W = caps_weights.sum(c) (64x128) and X = x.sum(p) (256x128), so the
kernel is a memory-bound reduction of x (151 MB) followed by a tiny
per-batch routing loop (matmuls of size <= 128x64x128).

Sharding: data-parallel over batch (dim 1 of x), 32 batch elements per
core; caps_weights replicated; no cross-core communication.

Per-core pipeline:
  phase 0: load caps_weights (64, 16*128), tree-reduce over c -> W (64,128).
  phase 1: for each of 9 p-tiles (128, 32*128=4096): HWDGE DMA to SBUF,
           8 fp32r ones-matmuls accumulate into 8 PSUM (1,512) banks.
           fp32r streams 1 col/cycle (vs 4 for fp32); the ones weight is
           exact, the moving operand is rounded (~1e-4 rel err on X).
  phase 2: X (1,4096) -> (32,128) via a DRAM bounce, PE transposes for
           layout flips, 3 routing iterations (o-on-partitions layout),
           softmax (b-on-partitions layout), output (32,128).
"""

import numpy as np

# ---- problem constants (hardcoded per contract) ----
P_TOT = 1152
BATCH = 256
O = 128
N_CAPS = 64
CAPS_DIM = 16
ITERATIONS = 3
N_CORES = 8
B_LOC = BATCH // N_CORES          # 32 batch elements per core
PT = P_TOT // 128                 # 9 p-tiles
FLAT = B_LOC * O                  # 4096 free elements per p-tile
NBLK = FLAT // 512                # 8 psum column blocks

_cache = {}


def _build():
    import concourse.bacc as bacc
    import concourse.tile as tile
    import concourse.mybir as mybir
    from concourse.masks import make_identity

    f32 = mybir.dt.float32
    f32r = mybir.dt.float32r
    AX = mybir.AxisListType
    AF = mybir.ActivationFunctionType
    OP = mybir.AluOpType

    nc = bacc.Bacc(None, target_bir_lowering=False)

    # x declared f32r: same bytes as fp32, lets plain HWDGE DMAs feed the
    # fast fp32r matmul path with no cast.
    x_in = nc.dram_tensor("x", [P_TOT, B_LOC, O], f32r, kind="ExternalInput")
    w_in = nc.dram_tensor("caps_weights", [N_CAPS, CAPS_DIM, O], f32,
                          kind="ExternalInput")
    cst_in = nc.dram_tensor("cst", [128, 2], f32r, kind="ExternalInput")
    out_d = nc.dram_tensor("out", [B_LOC, O], f32, kind="ExternalOutput")

    xv = x_in.rearrange("(t p) b o -> t p (b o)", p=128)   # (9, 128, 4096)

    with tile.TileContext(nc) as tc:
        with (
            tc.tile_pool(name="xin", bufs=PT) as xpool,
            tc.tile_pool(name="wrk", bufs=1) as wrk,
            tc.tile_pool(name="small", bufs=1) as small,
            tc.tile_pool(name="dram", bufs=1, space="DRAM") as dpool,
        ):
            # ---------------- phase 0: capsule weights + consts ----------
            w_sb = wrk.tile([N_CAPS, CAPS_DIM * O], f32)
            nc.sync.dma_start(w_sb[:], w_in.rearrange("n c o -> n (c o)"))
            t1 = wrk.tile([N_CAPS, 8 * O], f32)
            nc.vector.tensor_tensor(t1[:], w_sb[:, :8 * O], w_sb[:, 8 * O:], OP.add)
            t2 = wrk.tile([N_CAPS, 4 * O], f32)
            nc.vector.tensor_tensor(t2[:], t1[:, :4 * O], t1[:, 4 * O:], OP.add)
            t3 = wrk.tile([N_CAPS, 2 * O], f32)
            nc.vector.tensor_tensor(t3[:], t2[:, :2 * O], t2[:, 2 * O:], OP.add)
            w_no = wrk.tile([N_CAPS, O], f32)          # W[n,o]
            nc.vector.tensor_tensor(w_no[:], t3[:, :O], t3[:, O:], OP.add)

            ident = small.tile([128, 128], f32)
            make_identity(nc, ident[:])

            onesr = small.tile([128, 2], f32r)          # fp32r ones column
            nc.sync.dma_start(onesr[:], cst_in[:])
            ones_f = small.tile([128, 2], f32)          # fp32 ones column
            nc.vector.memset(ones_f[:], 1.0)
            ones_row = small.tile([1, 128], f32)        # (1,128) ones row
            nc.vector.memset(ones_row[:], 1.0)

            # ---------------- phase 1: big reduction ----------------
            with tc.tile_pool(name="ps_x", bufs=1, space="PSUM") as ps_xp:
                ps_xred = [ps_xp.tile([1, 512], f32, tag=f"psx{j}",
                                      name=f"psx{j}")
                           for j in range(NBLK)]
                for t in range(PT):
                    xt = xpool.tile([128, FLAT], f32r, tag="xt")
                    nc.sync.dma_start(xt[:], xv[t])
                    for j in range(NBLK):
                        nc.tensor.matmul(ps_xred[j][:], onesr[:, 0:1],
                                         xt[:, j * 512:(j + 1) * 512],
                                         start=(t == 0), stop=(t == PT - 1))
                x_row = wrk.tile([1, FLAT], f32)        # X flat (1, 4096)
                for j in range(NBLK):
                    if j % 2 == 0:
                        nc.vector.tensor_copy(
                            x_row[0:1, j * 512:(j + 1) * 512], ps_xred[j][:])
                    else:
                        nc.scalar.copy(
                            x_row[0:1, j * 512:(j + 1) * 512], ps_xred[j][:])

            # ---------------- phase 2: routing ----------------
            with tc.tile_pool(name="ps2", bufs=1, space="PSUM") as ps2:
                # W^T via PE transpose (needs psum; done after phase 1
                # because phase 1 occupies all 8 banks)
                ps_wt = ps2.tile([O, N_CAPS], f32, tag="ps_wt")
                nc.tensor.transpose(ps_wt[:], w_no[:], ident[:N_CAPS, :N_CAPS])
                wt_on = wrk.tile([O, N_CAPS], f32)      # W^T[o,n]
                nc.vector.tensor_copy(wt_on[:], ps_wt[:])

                # X (1,4096) -> (32,128) via DRAM bounce, then transpose
                dscr = dpool.tile([1, FLAT], f32)
                nc.sync.dma_start(dscr[:], x_row[:])
                x32 = wrk.tile([B_LOC, O], f32)         # X[b,o]
                nc.sync.dma_start(
                    x32[:], dscr[:].rearrange("a (b o) -> (a b) o", b=B_LOC))
                ps_xt = ps2.tile([O, B_LOC], f32, tag="ps_xt")
                nc.tensor.transpose(ps_xt[:], x32[:], ident[:B_LOC, :B_LOC])
                xt_ob = wrk.tile([O, B_LOC], f32)       # X^T[o,b]
                nc.vector.tensor_copy(xt_ob[:], ps_xt[:])

                coeffsT = wrk.tile([N_CAPS, B_LOC], f32)  # coeffs^T[n,b]
                nc.vector.memset(coeffsT[:], 1.0 / N_CAPS)
                logits = wrk.tile([B_LOC, N_CAPS], f32)
                nc.vector.memset(logits[:], 0.0)

                uT = wrk.tile([O, B_LOC], f32)
                sq = wrk.tile([O, B_LOC], f32)
                norm1 = wrk.tile([1, B_LOC], f32)
                den1 = wrk.tile([1, B_LOC], f32)
                rden1 = wrk.tile([1, B_LOC], f32)
                scale1 = wrk.tile([1, B_LOC], f32)
                routedT = wrk.tile([O, B_LOC], f32)

                for it in range(ITERATIONS):
                    # S^T[o,b] = sum_n W[n,o] coeffsT[n,b]
                    ps_s = ps2.tile([O, B_LOC], f32, tag="ps_s")
                    nc.tensor.matmul(ps_s[:], w_no[:], coeffsT[:],
                                     start=True, stop=True)
                    nc.vector.tensor_tensor(uT[:], xt_ob[:], ps_s[:], OP.mult)
                    nc.vector.tensor_tensor(sq[:], uT[:], uT[:], OP.mult)
                    # nsq[b] = sum_o sq[o,b] (partition reduction via matmul)
                    ps_n = ps2.tile([1, B_LOC], f32, tag="ps_n")
                    nc.tensor.matmul(ps_n[:], ones_f[:, 0:1], sq[:],
                                     start=True, stop=True)
                    # scale = norm / (1 + nsq)
                    nc.scalar.sqrt(norm1[:], ps_n[:])
                    nc.vector.tensor_scalar_add(den1[:], ps_n[:], 1.0)
                    nc.vector.reciprocal(rden1[:], den1[:])
                    nc.vector.tensor_tensor(scale1[:], norm1[:], rden1[:], OP.mult)
                    # broadcast scale over partitions: ones(1,128)^T @ (1,32)
                    ps_bc = ps2.tile([O, B_LOC], f32, tag="ps_bc")
                    nc.tensor.matmul(ps_bc[:], ones_row[:], scale1[:],
                                     start=True, stop=True)
                    nc.vector.tensor_tensor(routedT[:], ps_bc[:], uT[:], OP.mult)

                    if it < ITERATIONS - 1:
                        # delta[b,n] = sum_o (routed*X)[o,b] Wt[o,n]
                        tT = wrk.tile([O, B_LOC], f32, tag="tT")
                        nc.vector.tensor_tensor(tT[:], routedT[:], xt_ob[:],
                                                OP.mult)
                        ps_d = ps2.tile([B_LOC, N_CAPS], f32, tag="ps_d")
                        nc.tensor.matmul(ps_d[:], tT[:], wt_on[:],
                                         start=True, stop=True)
                        nc.vector.tensor_tensor(logits[:], logits[:], ps_d[:],
                                                OP.add)
                        # softmax over n (free axis)
                        negmx = wrk.tile([B_LOC, 1], f32, tag="negmx")
                        nc.vector.tensor_reduce(negmx[:], logits[:], AX.X,
                                                OP.max, negate=True)
                        ex = wrk.tile([B_LOC, N_CAPS], f32, tag="ex")
                        ssum = wrk.tile([B_LOC, 1], f32, tag="ssum")
                        nc.scalar.activation(ex[:], logits[:], AF.Exp,
                                             bias=negmx[:], accum_out=ssum[:])
                        rsum = wrk.tile([B_LOC, 1], f32, tag="rsum")
                        nc.vector.reciprocal(rsum[:], ssum[:])
                        coeffs = wrk.tile([B_LOC, N_CAPS], f32, tag="coeffs")
                        nc.vector.tensor_scalar_mul(coeffs[:], ex[:], rsum[:])
                        ps_ct = ps2.tile([N_CAPS, B_LOC], f32, tag="ps_ct")
                        nc.tensor.transpose(ps_ct[:], coeffs[:],
                                            ident[:B_LOC, :B_LOC])
                        nc.vector.tensor_copy(coeffsT[:], ps_ct[:])
                    else:
                        # output = routed^T transposed -> (32, 128)
                        ps_o = ps2.tile([B_LOC, O], f32, tag="ps_o")
                        nc.tensor.transpose(ps_o[:], routedT[:], ident[:])
                        out_sb = wrk.tile([B_LOC, O], f32, tag="out_sb")
                        nc.vector.tensor_copy(out_sb[:], ps_o[:])
                        nc.sync.dma_start(out_d[:], out_sb[:])

    nc.compile()
    return nc


def run_with_results(x: np.ndarray, caps_weights: np.ndarray, **run_kwargs):
    """Run the SPMD kernel; returns (output (256,1,128), BassKernelResults)."""
    from concourse.bass_utils import run_bass_kernel_spmd

    if "nc" not in _cache:
        _cache["nc"] = _build()
    nc = _cache["nc"]

    x = np.ascontiguousarray(x, dtype=np.float32)
    caps_weights = np.ascontiguousarray(caps_weights, dtype=np.float32)
    cst = np.ones((128, 2), dtype=np.float32)

    in_maps = []
    for c in range(N_CORES):
        in_maps.append({
            "x": np.ascontiguousarray(x[:, c * B_LOC:(c + 1) * B_LOC, :]),
            "caps_weights": caps_weights,
            "cst": cst,
        })
    res = run_bass_kernel_spmd(nc, in_maps, core_ids=list(range(N_CORES)),
                               **run_kwargs)
    out = np.concatenate([res.results[c]["out"] for c in range(N_CORES)], axis=0)
    return out.reshape(BATCH, 1, O), res


def kernel(x: np.ndarray, caps_weights: np.ndarray) -> np.ndarray:
    out, _ = run_with_results(x, caps_weights)
    return out


# revision 7
# speedup vs baseline: 1.0772x; 1.0772x over previous
"""Trainium2 Bass kernel for nn_Capsule: capsule routing head.

Math: the einsum 'nco,pbo->bno' factorizes as xp[b,n,o] = W[n,o] * X[b,o]
with W = caps_weights.sum(c) (64x128) and X = x.sum(p) (256x128), so the
kernel is a memory-bound reduction of x (151 MB) followed by a tiny
per-batch routing loop (matmuls of size <= 128x64x128).

Sharding: data-parallel over batch (dim 1 of x), 32 batch elements per
core; caps_weights replicated; no cross-core communication.

Per-core pipeline:
  - 9 p-tiles of x (128, 4096) stream in via both HWDGE rings (sync +
    scalar engines), issued before everything else.
  - Reduction via fp32r matmuls with one-hot-column stationary matrices:
    for p-tile t and batch b, matmul(psum(32,128) +=
    E_b^T @ x_tile[:, b*128:(b+1)*128]) where E_b has ones in column b.
    All 288 matmuls accumulate into ONE psum bank; X lands directly as
    (32,128).  fp32r streams 1 col/cycle (4x faster than fp32); the
    stationary is exact 0/1, the moving operand is rounded (~1e-4 rel).
  - Routing in b-on-partitions layout: norms via DVE free-axis reduce,
    softmax over free axis, sqrt(q) computed as Exp(0.5*Ln(q)) so every
    activation lives in one ACT table (no mid-kernel table reloads; the
    table registry is pinned to 'natural_log_exp_and_others').
"""

import numpy as np

# ---- problem constants (hardcoded per contract) ----
P_TOT = 1152
BATCH = 256
O = 128
N_CAPS = 64
CAPS_DIM = 16
ITERATIONS = 3
N_CORES = 8
B_LOC = BATCH // N_CORES          # 32 batch elements per core
PT = P_TOT // 128                 # 9 p-tiles
FLAT = B_LOC * O                  # 4096 free elements per p-tile

_cache = {}


def _pin_act_table():
    """Force every ACT function onto the one table containing
    Exp+Ln+Square+Copy, so the kernel needs a single ACT_TABLE_LOAD."""
    import functools
    import concourse.hw_specs as hw_specs
    import concourse.bacc as bacc_mod

    if getattr(hw_specs.get_activation_tables, "_capsule_pinned", False):
        return
    orig = hw_specs.get_activation_tables

    @functools.cache
    def pinned(module_arch):
        tabs = orig(module_arch)
        keep = None
        for name, fns in tabs.items():
            names = {f.name for f in fns}
            if {"Exp", "Ln", "Square", "Copy", "Identity"} <= names:
                keep = name
                break
        if keep is None:
            return tabs
        return {n: (fns if n == keep else type(fns)()) for n, fns in tabs.items()}

    pinned._capsule_pinned = True
    hw_specs.get_activation_tables = pinned
    bacc_mod.get_activation_tables = pinned


def _build():
    _pin_act_table()
    import concourse.bacc as bacc
    import concourse.tile as tile
    import concourse.mybir as mybir
    from concourse.masks import make_identity

    f32 = mybir.dt.float32
    f32r = mybir.dt.float32r
    AX = mybir.AxisListType
    AF = mybir.ActivationFunctionType
    OP = mybir.AluOpType

    nc = bacc.Bacc(None, target_bir_lowering=False)

    # x declared f32r: same bytes as fp32, lets plain HWDGE DMAs feed the
    # fast fp32r matmul path with no cast.
    x_in = nc.dram_tensor("x", [P_TOT, B_LOC, O], f32r, kind="ExternalInput")
    w_in = nc.dram_tensor("caps_weights", [N_CAPS, CAPS_DIM, O], f32,
                          kind="ExternalInput")
    # one-hot stationary source: (128, 63) with ones in column 31, so
    # cst[:, 31-b : 63-b] is the one-hot-column-b matrix E_b.
    cst_in = nc.dram_tensor("cst", [128, 2 * B_LOC - 1], f32r,
                            kind="ExternalInput")
    out_d = nc.dram_tensor("out", [B_LOC, O], f32, kind="ExternalOutput")

    xv = x_in.rearrange("(t p) b o -> t p (b o)", p=128)   # (9, 128, 4096)

    with tile.TileContext(nc) as tc:
        with (
            tc.tile_pool(name="xin", bufs=PT) as xpool,
            tc.tile_pool(name="wrk", bufs=1) as wrk,
            tc.tile_pool(name="small", bufs=1) as small,
            tc.tile_pool(name="ps", bufs=1, space="PSUM") as ps,
        ):
            # ---- DMAs first: x across both HWDGE rings; w/cst lead each ----
            zpat = small.tile([128, 2 * B_LOC - 1], f32r)
            nc.sync.dma_start(zpat[:], cst_in[:])
            w_sb = wrk.tile([N_CAPS, CAPS_DIM * O], f32)
            nc.scalar.dma_start(w_sb[:], w_in.rearrange("n c o -> n (c o)"))
            xts = []
            for t in range(PT):
                xt = xpool.tile([128, FLAT], f32r, tag="xt", name=f"xt{t}")
                eng = nc.sync if t % 2 == 0 else nc.scalar
                eng.dma_start(xt[:], xv[t])
                xts.append(xt)

            # ---- capsule weight prep (overlaps the x stream) ----
            t1 = wrk.tile([N_CAPS, 8 * O], f32)
            nc.vector.tensor_tensor(t1[:], w_sb[:, :8 * O], w_sb[:, 8 * O:], OP.add)
            t2 = wrk.tile([N_CAPS, 4 * O], f32)
            nc.vector.tensor_tensor(t2[:], t1[:, :4 * O], t1[:, 4 * O:], OP.add)
            t3 = wrk.tile([N_CAPS, 2 * O], f32)
            nc.vector.tensor_tensor(t3[:], t2[:, :2 * O], t2[:, 2 * O:], OP.add)
            w_no = wrk.tile([N_CAPS, O], f32)          # W[n,o]
            nc.vector.tensor_tensor(w_no[:], t3[:, :O], t3[:, O:], OP.add)

            ident = small.tile([128, 128], f32)
            make_identity(nc, ident[:])

            ps_wt = ps.tile([O, N_CAPS], f32, tag="ps_wt")
            nc.tensor.transpose(ps_wt[:], w_no[:], ident[:N_CAPS, :N_CAPS])
            wt_on = wrk.tile([O, N_CAPS], f32)          # W^T[o,n]
            nc.vector.tensor_copy(wt_on[:], ps_wt[:])

            # ---- reduction: X[b,o] = sum_p x[p,b,o] into one psum bank ----
            ps_x = ps.tile([B_LOC, O], f32, tag="ps_x")
            first = True
            for t in range(PT):
                for b in range(B_LOC):
                    nc.tensor.matmul(
                        ps_x[:], zpat[:, B_LOC - 1 - b: 2 * B_LOC - 1 - b],
                        xts[t][:, b * O:(b + 1) * O],
                        start=first, stop=(t == PT - 1 and b == B_LOC - 1),
                        skip_group_check=True)
                    first = False
            x32 = wrk.tile([B_LOC, O], f32)             # X[b,o]
            nc.vector.tensor_copy(x32[:], ps_x[:])

            # ---- routing (b on partitions) ----
            coeffsT = wrk.tile([N_CAPS, B_LOC], f32)    # coeffs^T[n,b]
            nc.vector.memset(coeffsT[:], 1.0 / N_CAPS)
            logits = wrk.tile([B_LOC, N_CAPS], f32)
            nc.vector.memset(logits[:], 0.0)

            u = wrk.tile([B_LOC, O], f32)
            sq = wrk.tile([B_LOC, O], f32)
            nsq = wrk.tile([B_LOC, 1], f32)
            lnq = wrk.tile([B_LOC, 1], f32)
            norm = wrk.tile([B_LOC, 1], f32)
            den = wrk.tile([B_LOC, 1], f32)
            rden = wrk.tile([B_LOC, 1], f32)
            scale = wrk.tile([B_LOC, 1], f32)

            for it in range(ITERATIONS):
                # S[b,o] = sum_n coeffsT[n,b] W[n,o]
                ps_s = ps.tile([B_LOC, O], f32, tag="ps_s")
                nc.tensor.matmul(ps_s[:], coeffsT[:], w_no[:],
                                 start=True, stop=True)
                nc.vector.tensor_tensor(u[:], x32[:], ps_s[:], OP.mult)
                # nsq = sum_o u^2 (free-axis); Square+accum on ACT
                nc.scalar.activation(sq[:], u[:], AF.Square, accum_out=nsq[:])
                # scale = sqrt(q)/(1+q); sqrt(q) = Exp(0.5*Ln(q))
                nc.scalar.activation(lnq[:], nsq[:], AF.Ln)
                nc.scalar.activation(norm[:], lnq[:], AF.Exp, scale=0.5)
                nc.vector.tensor_scalar_add(den[:], nsq[:], 1.0)
                nc.vector.reciprocal(rden[:], den[:])
                nc.vector.tensor_tensor(scale[:], norm[:], rden[:], OP.mult)

                if it < ITERATIONS - 1:
                    # t = routed*X = scale*u*X ; delta[b,n] = sum_o t W^T
                    ux = wrk.tile([B_LOC, O], f32, tag="ux")
                    nc.vector.tensor_tensor(ux[:], u[:], x32[:], OP.mult)
                    tb = wrk.tile([B_LOC, O], f32, tag="tb")
                    nc.vector.tensor_scalar_mul(tb[:], ux[:], scale[:])
                    ps_t = ps.tile([O, B_LOC], f32, tag="ps_t")
                    nc.tensor.transpose(ps_t[:], tb[:], ident[:B_LOC, :B_LOC])
                    tT = wrk.tile([O, B_LOC], f32, tag="tT")
                    nc.vector.tensor_copy(tT[:], ps_t[:])
                    ps_d = ps.tile([B_LOC, N_CAPS], f32, tag="ps_d")
                    nc.tensor.matmul(ps_d[:], tT[:], wt_on[:],
                                     start=True, stop=True)
                    nc.vector.tensor_tensor(logits[:], logits[:], ps_d[:],
                                            OP.add)
                    # softmax over n (free axis); logits are O(10), exp-safe
                    ex = wrk.tile([B_LOC, N_CAPS], f32, tag="ex")
                    ssum = wrk.tile([B_LOC, 1], f32, tag="ssum")
                    nc.scalar.activation(ex[:], logits[:], AF.Exp,
                                         accum_out=ssum[:])
                    rsum = wrk.tile([B_LOC, 1], f32, tag="rsum")
                    nc.vector.reciprocal(rsum[:], ssum[:])
                    coeffs = wrk.tile([B_LOC, N_CAPS], f32, tag="coeffs")
                    nc.vector.tensor_scalar_mul(coeffs[:], ex[:], rsum[:])
                    ps_ct = ps.tile([N_CAPS, B_LOC], f32, tag="ps_ct")
                    nc.tensor.transpose(ps_ct[:], coeffs[:],
                                        ident[:B_LOC, :B_LOC])
                    nc.vector.tensor_copy(coeffsT[:], ps_ct[:])
                else:
                    out_sb = wrk.tile([B_LOC, O], f32, tag="out_sb")
                    nc.vector.tensor_scalar_mul(out_sb[:], u[:], scale[:])
                    nc.sync.dma_start(out_d[:], out_sb[:])

    nc.compile()
    return nc


def run_with_results(x: np.ndarray, caps_weights: np.ndarray, **run_kwargs):
    """Run the SPMD kernel; returns (output (256,1,128), BassKernelResults)."""
    from concourse.bass_utils import run_bass_kernel_spmd

    if "nc" not in _cache:
        _cache["nc"] = _build()
    nc = _cache["nc"]

    x = np.ascontiguousarray(x, dtype=np.float32)
    caps_weights = np.ascontiguousarray(caps_weights, dtype=np.float32)
    cst = np.zeros((128, 2 * B_LOC - 1), dtype=np.float32)
    cst[:, B_LOC - 1] = 1.0

    in_maps = []
    for c in range(N_CORES):
        in_maps.append({
            "x": np.ascontiguousarray(x[:, c * B_LOC:(c + 1) * B_LOC, :]),
            "caps_weights": caps_weights,
            "cst": cst,
        })
    res = run_bass_kernel_spmd(nc, in_maps, core_ids=list(range(N_CORES)),
                               **run_kwargs)
    out = np.concatenate([res.results[c]["out"] for c in range(N_CORES)], axis=0)
    return out.reshape(BATCH, 1, O), res


def kernel(x: np.ndarray, caps_weights: np.ndarray) -> np.ndarray:
    out, _ = run_with_results(x, caps_weights)
    return out
